# revision 1
# baseline (speedup 1.0000x reference)
"""Teacher-forced Elman RNN decoder on 8 Trainium2 NeuronCores.

Math per time step t (T=512 steps, serial recurrence):
    h = tanh(x_t @ W_ih.T + b_ih + h @ W_hh.T + b_hh)     # [B, H], H=1024
    y_t = h @ W_out.T + b_out                              # [B, 1]

Sharding: data-parallel over batch B=256 -> 32 per core, weights replicated,
recurrence local to each core. No collectives.

Per-core kernel design (all operands SBUF-resident, zero per-step HBM traffic):
  * State kept TRANSPOSED: hT[p, k*32+b] = h[b, k*128+p]  (k = 0..7 H-chunks).
  * Main matmuls (layout "W stationary"): for output chunk o, contraction
    chunk k:  psum[o] += Wt(k,o).T @ hT_k, with Wt(k,o)[p, m] =
    W_hh[o*128+m, k*128+p].  Output is h_nextT directly -> no transposes.
  * The rank-1 input term + biases are SEEDED into PSUM ahead of time with a
    tiny K=2 matmul:  seed[m, r*32+b] = W_ih[om]*x_{t0+r}[b] + (b_ih+b_hh)[om]
    batched over 8 future steps (rhs = [x; 1] rows, N=256).  Main matmuls then
    accumulate on top (start=False), and ACT applies tanh straight out of PSUM.
  * y-projection runs on the (otherwise idle) vector engine: per step
    z[p, b] = sum_o W_out[o*128+p] * hT[p, o*32+b]  (mul + reduce), staged to
    DRAM; a final ones-vector matmul reduces over partitions.
  * PSUM: 8 banks = 2 groups (even/odd 8-step phase) x 4 banks; chunk o of
    phase q lives in bank q*4 + o//2, half-bank (o%2)*256, step slot (t%8)*32.

Hardware loop: For_i over NITER iterations of U=32 unrolled steps.
"""

import numpy as np

import concourse.bass as bass
import concourse.bacc as bacc
import concourse.mybir as mybir
import concourse.tile as tile
from concourse.bass_utils import run_bass_kernel_spmd

P = 128          # partitions
B = 32           # local batch (256 / 8 cores)
H = 1024
NCH = 8          # H / P chunks
T_FULL = 512
U = 64           # steps per hardware-loop iteration
N_CORES = 8

# dtype of W_hh tiles and hT state (psum accumulation is always fp32).
# fp32 matmul is 4 cycles/row on TRN2 (hi/lo emulation); fp16 runs full rate
# with fast-weight-load and has 11-bit mantissa (4x better than bf16).
DT_MAIN = mybir.dt.float16
NP_MAIN = np.float16

_CACHE = {}


def _build(t_total: int, debug: bool):
    """Build the Bass program for one core. Returns nc."""
    key = (t_total, debug, DT_MAIN)
    if key in _CACHE:
        return _CACHE[key]

    assert t_total % U == 0
    niter = t_total // U
    f32 = mybir.dt.float32

    nc = bacc.Bacc("TRN2", target_bir_lowering=False, debug=debug)

    # ---- DRAM I/O ----
    w_d = nc.dram_tensor("w", [P, NCH * NCH * P], DT_MAIN, kind="ExternalInput")
    # seed lhsT per bank a: rows [W_ih_2a; c_2a; W_ih_{2a+1}; c_{2a+1}]
    seedw_d = nc.dram_tensor("seedw", [4, 4 * P], DT_MAIN, kind="ExternalInput")
    # x rows: [x_t(b); ones], flattened t*32+b, padded one extra seed group
    x_d = nc.dram_tensor("x", [2, t_total * B + 512], DT_MAIN, kind="ExternalInput")
    h0_d = nc.dram_tensor("h0", [H, B], DT_MAIN, kind="ExternalInput")
    # wout pre-broadcast to the hT layout: wout[p, o*32+b] = W_out[o*128+p]
    wout_d = nc.dram_tensor("wout", [P, NCH * B], f32, kind="ExternalInput")
    y_d = nc.dram_tensor("y", [1, t_total * B], f32, kind="ExternalOutput")
    zbuf_d = nc.dram_tensor("zbuf", [P, t_total * B], f32)  # internal scratch

    with tile.TileContext(nc) as tc:
        with (
            tc.tile_pool(name="pool", bufs=1) as pool,
            tc.tile_pool(name="psum", bufs=1, space=bass.MemorySpace.PSUM) as psum,
        ):
            w_sb = pool.tile([P, NCH * NCH * P], DT_MAIN, tag="w")
            seedw_sb = pool.tile([4, 4 * P], DT_MAIN, tag="seedw")
            wout_sb = pool.tile([P, NCH * B], f32, tag="wout")
            hT = [
                pool.tile([P, NCH * B], DT_MAIN, tag="hT0", name="hT0"),
                pool.tile([P, NCH * B], DT_MAIN, tag="hT1", name="hT1"),
            ]
            prod = pool.tile([P, NCH * B], f32, tag="prod")
            # z staging ring: 32 step slots in-loop; 4 x 512 slots in epilogue
            z_ring = pool.tile([P, 2048], f32, tag="zring")
            y_sb = pool.tile([1, 4 * 512], f32, tag="ysb")
            # seed rhs [4, 512] per slot: [[x8, 0], [1, 0], [0, x8], [0, 1]]
            x_first = pool.tile([4, 512], DT_MAIN, tag="xfirst")
            x_ring = pool.tile([4, 4 * 512], DT_MAIN, tag="xring")
            ones_sb = pool.tile([P, 1], f32, tag="ones")

            ps = [
                psum.tile([P, 512], f32, tag=f"ps{i}", name=f"ps{i}")
                for i in range(8)
            ]

            def seed_mms(bank_par, rhs_ap):
                """Seed psum phase `bank_par` (8 future steps, whole banks)
                with W_ih (x) x + (b_ih + b_hh).  One start=True matmul per
                bank covering the full 2KB zero region."""
                for a in range(4):
                    nc.tensor.matmul(
                        ps[bank_par * 4 + a][:, 0:512],
                        seedw_sb[:, a * P:(a + 1) * P],
                        rhs_ap,
                        start=True,
                        stop=False,
                        skip_group_check=True,
                    )

            def fill_seed_rhs(dst_col0, src_ap_cols):
                """Fill one [4,512] seed-rhs slot from x rows [x;1]."""
                nc.sync.dma_start(
                    out=x_ring[0:2, dst_col0:dst_col0 + 256],
                    in_=src_ap_cols,
                )
                nc.sync.dma_start(
                    out=x_ring[2:4, dst_col0 + 256:dst_col0 + 512],
                    in_=src_ap_cols,
                )

            # ---- prologue ----
            nc.sync.dma_start(out=w_sb[:], in_=w_d[:])
            nc.sync.dma_start(out=seedw_sb[:], in_=seedw_d[:])
            nc.sync.dma_start(out=wout_sb[:], in_=wout_d[:])
            nc.sync.dma_start(
                out=hT[0][:].rearrange("p (k b) -> p k b", k=NCH),
                in_=h0_d[:].rearrange("(k p) b -> p k b", p=P),
            )
            nc.gpsimd.memset(ones_sb[:], 1.0)
            nc.gpsimd.memset(x_first[:], 0.0)
            nc.gpsimd.memset(x_ring[:], 0.0)
            nc.sync.dma_start(out=x_first[0:2, 0:256], in_=x_d[:, 0:256])
            nc.sync.dma_start(out=x_first[2:4, 256:512], in_=x_d[:, 0:256])
            fill_seed_rhs(0, x_d[:, 256:512])
            seed_mms(0, x_first[:])  # seeds for steps 0..7

            # ---- main loop ----
            with tc.For_i(0, niter, 1, hint_engines=(mybir.EngineType.PE,)) as g:
                for j in range(U):
                    q = (j // 8) % 2          # psum phase of this step
                    r = j % 8                 # slot within phase
                    h_in = hT[j % 2]
                    h_out = hT[(j + 1) % 2]

                    if j % 8 == 0:
                        # seed psum for steps 8 ahead (opposite phase)
                        s = (j // 8) % 4
                        seed_mms(1 - q, x_ring[:, s * 512:(s + 1) * 512])
                    if j % 8 == 4:
                        # prefetch x for the seed group after the next one
                        s_next = (j // 8 + 1) % 4
                        gg = g * ((U // 8) * 256) + (j // 8 + 2) * 256
                        fill_seed_rhs(s_next * 512, x_d[:, bass.ds(gg, 256)])

                    # h_preT[o] += sum_k Wt(k,o).T @ hT_k
                    # Chunk pairs (2a, 2a+1) share a bank. Pair emission is
                    # rotated by step and each pair consumes hT chunks in the
                    # previous step's production order, so every cross-step
                    # ACT->matmul edge has >= ~16 matmul slots of slack and
                    # PE never stalls on tanh completion.
                    k_order = [
                        2 * ((j - 1 + i2) % 4) + h2
                        for i2 in range(4) for h2 in range(2)
                    ]
                    for i in range(4):
                        a = (j + i) % 4
                        bank = ps[q * 4 + a]
                        for k in k_order:
                            for half in range(2):
                                o = 2 * a + half
                                nc.tensor.matmul(
                                    bank[:, half * 256 + r * 32:half * 256 + r * 32 + 32],
                                    w_sb[:, (k * NCH + o) * P:(k * NCH + o + 1) * P],
                                    h_in[:, k * 32:(k + 1) * 32],
                                    start=False,
                                    stop=(k == k_order[-1]),
                                    skip_group_check=True,
                                )
                        # tanh the whole pair straight out of psum
                        nc.scalar.activation(
                            h_out[:].rearrange("p (o b) -> p o b", b=B)[
                                :, 2 * a:2 * a + 2, :],
                            bank[:].rearrange("p (h rb) -> p h rb", h=2)[
                                :, :, r * 32:r * 32 + 32],
                            mybir.ActivationFunctionType.Tanh,
                        )

                    # z[p, b] = sum_o wout[p, o*32+b] * h_out[p, o*32+b]
                    # (mult fully contiguous; only the reduce is strided)
                    nc.vector.tensor_mul(prod[:], wout_sb[:], h_out[:])
                    nc.vector.tensor_reduce(
                        z_ring[:, (j % 64) * 32:(j % 64) * 32 + 32],
                        prod[:].rearrange("p (o b) -> p b o", o=NCH),
                        axis=mybir.AxisListType.X,
                        op=mybir.AluOpType.add,
                    )

                    if j % 16 == 15:
                        zslot = ((j // 16) % 4) * 512
                        nc.sync.dma_start(
                            out=zbuf_d[:, bass.ds(g * (U * B) + (j // 16) * 512, 512)],
                            in_=z_ring[:, zslot:zslot + 512],
                        )

            # ---- epilogue: y[t*32+b] = sum_p zbuf[p, t*32+b] ----
            nchunks = t_total * B // 512
            for c in range(nchunks):
                slot = c % 4
                nc.sync.dma_start(
                    out=z_ring[:, slot * 512:(slot + 1) * 512],
                    in_=zbuf_d[:, c * 512:(c + 1) * 512],
                )
                nc.tensor.matmul(
                    ps[c % 8][0:1, 0:512],
                    ones_sb[:, 0:1],
                    z_ring[:, slot * 512:(slot + 1) * 512],
                    start=True,
                    stop=True,
                )
                yslot = c % 4
                if c % 2 == 0:
                    nc.vector.tensor_copy(
                        y_sb[0:1, yslot * 512:(yslot + 1) * 512],
                        ps[c % 8][0:1, 0:512],
                    )
                else:
                    nc.scalar.copy(
                        y_sb[0:1, yslot * 512:(yslot + 1) * 512],
                        ps[c % 8][0:1, 0:512],
                    )
                nc.sync.dma_start(
                    out=y_d[0:1, c * 512:(c + 1) * 512],
                    in_=y_sb[0:1, yslot * 512:(yslot + 1) * 512],
                )

    nc.compile()
    _CACHE[key] = nc
    return nc


def _prep_inputs(initial_input, hidden, targets, W_ih, W_hh, b_ih, b_hh,
                 W_out, t_total):
    """Host-side prep: returns the 8 per-core input maps."""
    f32 = np.float32
    # x sequence: teacher-forced input at step t is targets[t-1], x_0 = initial
    x_seq = np.concatenate(
        [np.asarray(initial_input, f32).reshape(1, -1),
         np.asarray(targets, f32)[: t_total - 1, :, 0]],
        axis=0,
    )  # [T, 256]
    c_bias = (np.asarray(b_ih, f32) + np.asarray(b_hh, f32))
    # w_sb[p, (k*8+o)*128+m] = W_hh.T[k*128+p, o*128+m]
    wt = (
        np.asarray(W_hh, f32).T.reshape(NCH, P, NCH, P)
        .transpose(1, 0, 2, 3)
        .reshape(P, NCH * NCH * P)
        .astype(NP_MAIN)
    )
    # seedw[opar*2+t, a*128+m] = [W_ih; c][t] at H-index (2a+opar)*128+m
    sw = np.stack(
        [np.asarray(W_ih, f32)[:, 0].reshape(NCH, P), c_bias.reshape(NCH, P)],
        axis=1,
    )  # [o, t, m]
    seedw = (
        sw.reshape(4, 2, 2, P).transpose(1, 2, 0, 3).reshape(4, 4 * P)
        .astype(NP_MAIN)
    )
    wout8 = np.asarray(W_out, f32)[0].reshape(NCH, P).T                  # [128, 8]
    wout = np.ascontiguousarray(
        np.broadcast_to(wout8[:, :, None], (P, NCH, B)).reshape(P, NCH * B)
    )

    in_maps = []
    for ci in range(N_CORES):
        sl = slice(ci * B, (ci + 1) * B)
        xpad = np.zeros((2, t_total * B + 512), NP_MAIN)
        xpad[0, : t_total * B] = x_seq[:, sl].reshape(-1).astype(NP_MAIN)
        xpad[1, :] = 1.0
        h0 = np.ascontiguousarray(np.asarray(hidden, f32)[sl].T).astype(NP_MAIN)
        in_maps.append({
            "w": wt, "seedw": seedw, "x": xpad, "h0": h0, "wout": wout,
        })
    return in_maps


def kernel(initial_input, hidden, targets, W_ih, W_hh, b_ih, b_hh, W_out,
           b_out, teacher_force_probability=None, _trace=False):
    t_total = int(np.asarray(targets).shape[0])
    nc = _build(t_total, debug=False)
    in_maps = _prep_inputs(initial_input, hidden, targets, W_ih, W_hh, b_ih,
                           b_hh, W_out, t_total)
    res = run_bass_kernel_spmd(nc, in_maps, core_ids=list(range(N_CORES)),
                               trace=_trace)
    # y_core[0, t*32+b]; global batch index = core*32 + b
    y = np.concatenate(
        [r["y"].reshape(t_total, B) for r in res.results], axis=1
    ).astype(np.float32)
    y = y + np.float32(np.asarray(b_out).reshape(-1)[0])
    out = y[:, :, None]
    if _trace:
        return out, res
    return out



# revision 2
# speedup vs baseline: 1.5474x; 1.5474x over previous
"""Teacher-forced Elman RNN decoder on 8 Trainium2 NeuronCores.

Math per time step t (T=512 steps, serial recurrence):
    h = tanh(x_t @ W_ih.T + b_ih + h @ W_hh.T + b_hh)     # [B, H], H=1024
    y_t = h @ W_out.T + b_out                              # [B, 1]

Sharding: data-parallel over batch B=256 -> 32 per core, weights replicated,
recurrence local to each core. No collectives.

Per-core kernel design (all operands SBUF-resident, zero per-step HBM traffic):
  * State kept TRANSPOSED: hT[p, k*32+b] = h[b, k*128+p]  (k = 0..7 H-chunks).
  * Main matmuls (layout "W stationary"): for output chunk o, contraction
    chunk k:  psum[o] += Wt(k,o).T @ hT_k, with Wt(k,o)[p, m] =
    W_hh[o*128+m, k*128+p].  Output is h_nextT directly -> no transposes.
  * The rank-1 input term + biases are SEEDED into PSUM ahead of time with a
    tiny K=2 matmul:  seed[m, r*32+b] = W_ih[om]*x_{t0+r}[b] + (b_ih+b_hh)[om]
    batched over 8 future steps (rhs = [x; 1] rows, N=256).  Main matmuls then
    accumulate on top (start=False), and ACT applies tanh straight out of PSUM.
  * y-projection runs on the (otherwise idle) vector engine: per step
    z[p, b] = sum_o W_out[o*128+p] * hT[p, o*32+b]  (mul + reduce), staged to
    DRAM; a final ones-vector matmul reduces over partitions.
  * PSUM: 8 banks = 2 groups (even/odd 8-step phase) x 4 banks; chunk o of
    phase q lives in bank q*4 + o//2, half-bank (o%2)*256, step slot (t%8)*32.

Hardware loop: For_i over NITER iterations of U=32 unrolled steps.
"""

import numpy as np

import concourse.bass as bass
import concourse.bacc as bacc
import concourse.mybir as mybir
import concourse.tile as tile
from concourse.bass_utils import run_bass_kernel_spmd

P = 128          # partitions
B = 32           # local batch (256 / 8 cores)
H = 1024
NCH = 8          # H / P chunks
T_FULL = 512
U = 64           # steps per hardware-loop iteration
N_CORES = 8

# dtype of W_hh tiles and hT state (psum accumulation is always fp32).
# fp32 matmul is 4 cycles/row on TRN2 (hi/lo emulation); fp16 runs full rate
# with fast-weight-load and has 11-bit mantissa (4x better than bf16).
DT_MAIN = mybir.dt.float16
NP_MAIN = np.float16

_CACHE = {}


def _build(t_total: int, debug: bool, nrep: int = 1):
    """Build the Bass program for one core. Returns nc."""
    key = (t_total, debug, DT_MAIN, nrep)
    if key in _CACHE:
        return _CACHE[key]

    assert t_total % U == 0
    niter = t_total // U
    f32 = mybir.dt.float32

    nc = bacc.Bacc("TRN2", target_bir_lowering=False, debug=debug)

    # ---- DRAM I/O ----
    w_d = nc.dram_tensor("w", [P, NCH * NCH * P], DT_MAIN, kind="ExternalInput")
    # seed lhsT per bank a: rows [W_ih_2a; c_2a; W_ih_{2a+1}; c_{2a+1}]
    seedw_d = nc.dram_tensor("seedw", [4, 4 * P], DT_MAIN, kind="ExternalInput")
    # x rows: [x_t(b); ones], flattened t*32+b, padded one extra seed group
    x_d = nc.dram_tensor("x", [2, t_total * B + 512], DT_MAIN, kind="ExternalInput")
    h0_d = nc.dram_tensor("h0", [H, B], DT_MAIN, kind="ExternalInput")
    # wout pre-broadcast to the hT layout: wout[p, o*32+b] = W_out[o*128+p]
    wout_d = nc.dram_tensor("wout", [P, NCH * B], f32, kind="ExternalInput")
    y_d = nc.dram_tensor("y", [1, t_total * B], f32, kind="ExternalOutput")
    zbuf_d = nc.dram_tensor("zbuf", [P, t_total * B], f32)  # internal scratch

    with tile.TileContext(nc) as tc:
        with (
            tc.tile_pool(name="pool", bufs=1) as pool,
            tc.tile_pool(name="psum", bufs=1, space=bass.MemorySpace.PSUM) as psum,
        ):
            w_sb = pool.tile([P, NCH * NCH * P], DT_MAIN, tag="w")
            seedw_sb = pool.tile([4, 4 * P], DT_MAIN, tag="seedw")
            wout_sb = pool.tile([P, NCH * B], f32, tag="wout")
            hT = [
                pool.tile([P, NCH * B], DT_MAIN, tag="hT0", name="hT0"),
                pool.tile([P, NCH * B], DT_MAIN, tag="hT1", name="hT1"),
            ]
            prod = pool.tile([P, NCH * B], f32, tag="prod")
            # z staging ring: 32 step slots in-loop; 4 x 512 slots in epilogue
            z_ring = pool.tile([P, 2048], f32, tag="zring")
            y_sb = pool.tile([1, 4 * 512], f32, tag="ysb")
            # seed rhs [4, 512] per slot: [[x8, 0], [1, 0], [0, x8], [0, 1]]
            x_first = pool.tile([4, 512], DT_MAIN, tag="xfirst")
            x_ring = pool.tile([4, 4 * 512], DT_MAIN, tag="xring")
            ones_sb = pool.tile([P, 1], f32, tag="ones")

            ps = [
                psum.tile([P, 512], f32, tag=f"ps{i}", name=f"ps{i}")
                for i in range(8)
            ]

            def seed_mms(bank_par, rhs_ap):
                """Seed psum phase `bank_par` (8 future steps, whole banks)
                with W_ih (x) x + (b_ih + b_hh).  One start=True matmul per
                bank covering the full 2KB zero region."""
                for a in range(4):
                    nc.tensor.matmul(
                        ps[bank_par * 4 + a][:, 0:512],
                        seedw_sb[:, a * P:(a + 1) * P],
                        rhs_ap,
                        start=True,
                        stop=False,
                        skip_group_check=True,
                    )

            def fill_seed_rhs(dst_col0, src_ap_cols):
                """Fill one [4,512] seed-rhs slot from x rows [x;1]."""
                nc.sync.dma_start(
                    out=x_ring[0:2, dst_col0:dst_col0 + 256],
                    in_=src_ap_cols,
                )
                nc.sync.dma_start(
                    out=x_ring[2:4, dst_col0 + 256:dst_col0 + 512],
                    in_=src_ap_cols,
                )

            # ---- prologue ----
            nc.sync.dma_start(out=w_sb[:], in_=w_d[:])
            nc.sync.dma_start(out=seedw_sb[:], in_=seedw_d[:])
            nc.sync.dma_start(out=wout_sb[:], in_=wout_d[:])
            nc.sync.dma_start(
                out=hT[0][:].rearrange("p (k b) -> p k b", k=NCH),
                in_=h0_d[:].rearrange("(k p) b -> p k b", p=P),
            )
            nc.gpsimd.memset(ones_sb[:], 1.0)
            nc.gpsimd.memset(x_first[:], 0.0)
            nc.gpsimd.memset(x_ring[:], 0.0)
            nc.sync.dma_start(out=x_first[0:2, 0:256], in_=x_d[:, 0:256])
            nc.sync.dma_start(out=x_first[2:4, 256:512], in_=x_d[:, 0:256])
            fill_seed_rhs(0, x_d[:, 256:512])
            seed_mms(0, x_first[:])  # seeds for steps 0..7

            # ---- main loop (nrep>1: timing-only outer repeat) ----
            import contextlib
            with contextlib.ExitStack() as _stk:
                if nrep > 1:
                    _stk.enter_context(tc.For_i(0, nrep, 1, name='rep'))
                _ctx = tc.For_i(0, niter, 1, hint_engines=(mybir.EngineType.PE,))
                g = _stk.enter_context(_ctx)
                for j in range(U):
                    q = (j // 8) % 2          # psum phase of this step
                    r = j % 8                 # slot within phase
                    h_in = hT[j % 2]
                    h_out = hT[(j + 1) % 2]

                    if j % 8 == 0:
                        # seed psum for steps 8 ahead (opposite phase)
                        s = (j // 8) % 4
                        seed_mms(1 - q, x_ring[:, s * 512:(s + 1) * 512])
                    if j % 8 == 4:
                        # prefetch x for the seed group after the next one
                        s_next = (j // 8 + 1) % 4
                        gg = g * ((U // 8) * 256) + (j // 8 + 2) * 256
                        fill_seed_rhs(s_next * 512, x_d[:, bass.ds(gg, 256)])

                    # h_preT[o] += sum_k Wt(k,o).T @ hT_k
                    # Chunk pairs (2a, 2a+1) share a bank. Pair emission is
                    # rotated by step and each pair consumes hT chunks in the
                    # previous step's production order, so every cross-step
                    # ACT->matmul edge has >= ~16 matmul slots of slack and
                    # PE never stalls on tanh completion.
                    k_order = [
                        2 * ((j - 1 + i2) % 4) + h2
                        for i2 in range(4) for h2 in range(2)
                    ]
                    for i in range(4):
                        a = (j + i) % 4
                        bank = ps[q * 4 + a]
                        for k in k_order:
                            for half in range(2):
                                o = 2 * a + half
                                nc.tensor.matmul(
                                    bank[:, half * 256 + r * 32:half * 256 + r * 32 + 32],
                                    w_sb[:, (k * NCH + o) * P:(k * NCH + o + 1) * P],
                                    h_in[:, k * 32:(k + 1) * 32],
                                    start=False,
                                    stop=(k == k_order[-1]),
                                    skip_group_check=True,
                                )
                        # tanh the whole pair straight out of psum
                        nc.scalar.activation(
                            h_out[:].rearrange("p (o b) -> p o b", b=B)[
                                :, 2 * a:2 * a + 2, :],
                            bank[:].rearrange("p (h rb) -> p h rb", h=2)[
                                :, :, r * 32:r * 32 + 32],
                            mybir.ActivationFunctionType.Tanh,
                        )

                    # z[p, b] = sum_o wout[p, o*32+b] * h_out[p, o*32+b]
                    # (mult fully contiguous; only the reduce is strided)
                    nc.vector.tensor_mul(prod[:], wout_sb[:], h_out[:])
                    nc.vector.tensor_reduce(
                        z_ring[:, (j % 64) * 32:(j % 64) * 32 + 32],
                        prod[:].rearrange("p (o b) -> p b o", o=NCH),
                        axis=mybir.AxisListType.X,
                        op=mybir.AluOpType.add,
                    )

                    if j % 16 == 15:
                        zslot = ((j // 16) % 4) * 512
                        nc.sync.dma_start(
                            out=zbuf_d[:, bass.ds(g * (U * B) + (j // 16) * 512, 512)],
                            in_=z_ring[:, zslot:zslot + 512],
                        )

            # ---- epilogue: y[t*32+b] = sum_p zbuf[p, t*32+b] ----
            nchunks = t_total * B // 512
            for c in range(nchunks):
                slot = c % 4
                nc.sync.dma_start(
                    out=z_ring[:, slot * 512:(slot + 1) * 512],
                    in_=zbuf_d[:, c * 512:(c + 1) * 512],
                )
                nc.tensor.matmul(
                    ps[c % 8][0:1, 0:512],
                    ones_sb[:, 0:1],
                    z_ring[:, slot * 512:(slot + 1) * 512],
                    start=True,
                    stop=True,
                )
                yslot = c % 4
                if c % 2 == 0:
                    nc.vector.tensor_copy(
                        y_sb[0:1, yslot * 512:(yslot + 1) * 512],
                        ps[c % 8][0:1, 0:512],
                    )
                else:
                    nc.scalar.copy(
                        y_sb[0:1, yslot * 512:(yslot + 1) * 512],
                        ps[c % 8][0:1, 0:512],
                    )
                nc.sync.dma_start(
                    out=y_d[0:1, c * 512:(c + 1) * 512],
                    in_=y_sb[0:1, yslot * 512:(yslot + 1) * 512],
                )

    nc.compile()
    _CACHE[key] = nc
    return nc


def _prep_inputs(initial_input, hidden, targets, W_ih, W_hh, b_ih, b_hh,
                 W_out, t_total):
    """Host-side prep: returns the 8 per-core input maps."""
    f32 = np.float32
    # x sequence: teacher-forced input at step t is targets[t-1], x_0 = initial
    x_seq = np.concatenate(
        [np.asarray(initial_input, f32).reshape(1, -1),
         np.asarray(targets, f32)[: t_total - 1, :, 0]],
        axis=0,
    )  # [T, 256]
    c_bias = (np.asarray(b_ih, f32) + np.asarray(b_hh, f32))
    # w_sb[p, (k*8+o)*128+m] = W_hh.T[k*128+p, o*128+m]
    wt = (
        np.asarray(W_hh, f32).T.reshape(NCH, P, NCH, P)
        .transpose(1, 0, 2, 3)
        .reshape(P, NCH * NCH * P)
        .astype(NP_MAIN)
    )
    # seedw[opar*2+t, a*128+m] = [W_ih; c][t] at H-index (2a+opar)*128+m
    sw = np.stack(
        [np.asarray(W_ih, f32)[:, 0].reshape(NCH, P), c_bias.reshape(NCH, P)],
        axis=1,
    )  # [o, t, m]
    seedw = (
        sw.reshape(4, 2, 2, P).transpose(1, 2, 0, 3).reshape(4, 4 * P)
        .astype(NP_MAIN)
    )
    wout8 = np.asarray(W_out, f32)[0].reshape(NCH, P).T                  # [128, 8]
    wout = np.ascontiguousarray(
        np.broadcast_to(wout8[:, :, None], (P, NCH, B)).reshape(P, NCH * B)
    )

    in_maps = []
    for ci in range(N_CORES):
        sl = slice(ci * B, (ci + 1) * B)
        xpad = np.zeros((2, t_total * B + 512), NP_MAIN)
        xpad[0, : t_total * B] = x_seq[:, sl].reshape(-1).astype(NP_MAIN)
        xpad[1, :] = 1.0
        h0 = np.ascontiguousarray(np.asarray(hidden, f32)[sl].T).astype(NP_MAIN)
        in_maps.append({
            "w": wt, "seedw": seedw, "x": xpad, "h0": h0, "wout": wout,
        })
    return in_maps


def kernel(initial_input, hidden, targets, W_ih, W_hh, b_ih, b_hh, W_out,
           b_out, teacher_force_probability=None, _trace=False):
    t_total = int(np.asarray(targets).shape[0])
    nc = _build(t_total, debug=False)
    in_maps = _prep_inputs(initial_input, hidden, targets, W_ih, W_hh, b_ih,
                           b_hh, W_out, t_total)
    res = run_bass_kernel_spmd(nc, in_maps, core_ids=list(range(N_CORES)),
                               trace=_trace)
    # y_core[0, t*32+b]; global batch index = core*32 + b
    y = np.concatenate(
        [r["y"].reshape(t_total, B) for r in res.results], axis=1
    ).astype(np.float32)
    y = y + np.float32(np.asarray(b_out).reshape(-1)[0])
    out = y[:, :, None]
    if _trace:
        return out, res
    return out



# revision 3
# speedup vs baseline: 1.6390x; 1.0592x over previous
"""Teacher-forced Elman RNN decoder on 8 Trainium2 NeuronCores.

Math per time step t (T=512 steps, serial recurrence):
    h = tanh(x_t @ W_ih.T + b_ih + h @ W_hh.T + b_hh)     # [B, H], H=1024
    y_t = h @ W_out.T + b_out                              # [B, 1]

Sharding: data-parallel over batch B=256 -> 32 per core, weights replicated,
recurrence local to each core. No collectives.

Per-core kernel design (all operands SBUF-resident, zero per-step HBM traffic):
  * State kept TRANSPOSED: hT[p, k*32+b] = h[b, k*128+p]  (k = 0..7 H-chunks).
  * Main matmuls (layout "W stationary"): for output chunk o, contraction
    chunk k:  psum[o] += Wt(k,o).T @ hT_k, with Wt(k,o)[p, m] =
    W_hh[o*128+m, k*128+p].  Output is h_nextT directly -> no transposes.
  * The rank-1 input term + biases are SEEDED into PSUM ahead of time with a
    tiny K=2 matmul:  seed[m, r*32+b] = W_ih[om]*x_{t0+r}[b] + (b_ih+b_hh)[om]
    batched over 8 future steps (rhs = [x; 1] rows, N=256).  Main matmuls then
    accumulate on top (start=False), and ACT applies tanh straight out of PSUM.
  * y-projection runs on the (otherwise idle) vector engine: per step
    z[p, b] = sum_o W_out[o*128+p] * hT[p, o*32+b]  (mul + reduce), staged to
    DRAM; a final ones-vector matmul reduces over partitions.
  * PSUM: 8 banks = 2 groups (even/odd 8-step phase) x 4 banks; chunk o of
    phase q lives in bank q*4 + o//2, half-bank (o%2)*256, step slot (t%8)*32.

Hardware loop: For_i over NITER iterations of U=32 unrolled steps.
"""

import numpy as np

import concourse.bass as bass
import concourse.bacc as bacc
import concourse.mybir as mybir
import concourse.tile as tile
from concourse.bass_utils import run_bass_kernel_spmd

P = 128          # partitions
B = 32           # local batch (256 / 8 cores)
H = 1024
NCH = 8          # H / P chunks
T_FULL = 512
U = 128          # steps per hardware-loop iteration
N_CORES = 8

# dtype of W_hh tiles and hT state (psum accumulation is always fp32).
# fp32 matmul is 4 cycles/row on TRN2 (hi/lo emulation); fp16 runs full rate
# with fast-weight-load and has 11-bit mantissa (4x better than bf16).
DT_MAIN = mybir.dt.float16
NP_MAIN = np.float16

_CACHE = {}


def _build(t_total: int, debug: bool, nrep: int = 1):
    """Build the Bass program for one core. Returns nc."""
    key = (t_total, debug, DT_MAIN, nrep)
    if key in _CACHE:
        return _CACHE[key]

    assert t_total % U == 0
    niter = t_total // U
    f32 = mybir.dt.float32

    nc = bacc.Bacc("TRN2", target_bir_lowering=False, debug=debug)

    # ---- DRAM I/O ----
    w_d = nc.dram_tensor("w", [P, NCH * NCH * P], DT_MAIN, kind="ExternalInput")
    # seed lhsT per bank a: rows [W_ih_2a; c_2a; W_ih_{2a+1}; c_{2a+1}]
    seedw_d = nc.dram_tensor("seedw", [4, 4 * P], DT_MAIN, kind="ExternalInput")
    # x rows: [x_t(b); ones], flattened t*32+b, padded one extra seed group
    x_d = nc.dram_tensor("x", [2, t_total * B + 512], DT_MAIN, kind="ExternalInput")
    h0_d = nc.dram_tensor("h0", [H, B], DT_MAIN, kind="ExternalInput")
    # wout pre-broadcast to the hT layout: wout[p, o*32+b] = W_out[o*128+p]
    wout_d = nc.dram_tensor("wout", [P, NCH * B], f32, kind="ExternalInput")
    y_d = nc.dram_tensor("y", [1, t_total * B], f32, kind="ExternalOutput")
    zbuf_d = nc.dram_tensor("zbuf", [P, t_total * B], f32)  # internal scratch

    with tile.TileContext(nc) as tc:
        with (
            tc.tile_pool(name="pool", bufs=1) as pool,
            tc.tile_pool(name="psum", bufs=1, space=bass.MemorySpace.PSUM) as psum,
        ):
            w_sb = pool.tile([P, NCH * NCH * P], DT_MAIN, tag="w")
            seedw_sb = pool.tile([4, 4 * P], DT_MAIN, tag="seedw")
            wout_sb = pool.tile([P, NCH * B], f32, tag="wout")
            hT = [
                pool.tile([P, NCH * B], DT_MAIN, tag="hT0", name="hT0"),
                pool.tile([P, NCH * B], DT_MAIN, tag="hT1", name="hT1"),
            ]
            prod = pool.tile([P, NCH * B], f32, tag="prod")
            # z staging ring: 32 step slots in-loop; 4 x 512 slots in epilogue
            z_ring = pool.tile([P, 2048], f32, tag="zring")
            y_sb = pool.tile([1, 4 * 512], f32, tag="ysb")
            # seed rhs [4, 512] per slot: [[x8, 0], [1, 0], [0, x8], [0, 1]]
            x_first = pool.tile([4, 512], DT_MAIN, tag="xfirst")
            x_ring = pool.tile([4, 4 * 512], DT_MAIN, tag="xring")
            ones_sb = pool.tile([P, 1], f32, tag="ones")

            ps = [
                psum.tile([P, 512], f32, tag=f"ps{i}", name=f"ps{i}")
                for i in range(8)
            ]

            def seed_mms(bank_par, rhs_ap):
                """Seed psum phase `bank_par` (8 future steps, whole banks)
                with W_ih (x) x + (b_ih + b_hh).  One start=True matmul per
                bank covering the full 2KB zero region."""
                for a in range(4):
                    nc.tensor.matmul(
                        ps[bank_par * 4 + a][:, 0:512],
                        seedw_sb[:, a * P:(a + 1) * P],
                        rhs_ap,
                        start=True,
                        stop=False,
                        skip_group_check=True,
                    )

            def fill_seed_rhs(dst_col0, src_ap_cols):
                """Fill one [4,512] seed-rhs slot from x rows [x;1]."""
                nc.sync.dma_start(
                    out=x_ring[0:2, dst_col0:dst_col0 + 256],
                    in_=src_ap_cols,
                )
                nc.sync.dma_start(
                    out=x_ring[2:4, dst_col0 + 256:dst_col0 + 512],
                    in_=src_ap_cols,
                )

            # ---- prologue ----
            nc.sync.dma_start(out=w_sb[:], in_=w_d[:])
            nc.sync.dma_start(out=seedw_sb[:], in_=seedw_d[:])
            nc.sync.dma_start(out=wout_sb[:], in_=wout_d[:])
            nc.sync.dma_start(
                out=hT[0][:].rearrange("p (k b) -> p k b", k=NCH),
                in_=h0_d[:].rearrange("(k p) b -> p k b", p=P),
            )
            nc.gpsimd.memset(ones_sb[:], 1.0)
            nc.gpsimd.memset(x_first[:], 0.0)
            nc.gpsimd.memset(x_ring[:], 0.0)
            nc.sync.dma_start(out=x_first[0:2, 0:256], in_=x_d[:, 0:256])
            nc.sync.dma_start(out=x_first[2:4, 256:512], in_=x_d[:, 0:256])
            fill_seed_rhs(0, x_d[:, 256:512])
            seed_mms(0, x_first[:])  # seeds for steps 0..7

            # ---- main loop (nrep>1: timing-only outer repeat) ----
            import contextlib
            with contextlib.ExitStack() as _stk:
                if nrep > 1:
                    _stk.enter_context(tc.For_i(0, nrep, 1, name='rep'))
                _ctx = tc.For_i(0, niter, 1, hint_engines=(mybir.EngineType.PE,))
                g = _stk.enter_context(_ctx)
                for j in range(U):
                    q = (j // 8) % 2          # psum phase of this step
                    r = j % 8                 # slot within phase
                    h_in = hT[j % 2]
                    h_out = hT[(j + 1) % 2]

                    if j % 8 == 1:
                        # seed psum for steps 7 ahead (opposite phase); one
                        # step late so the WAR wait on the prior phase's
                        # tanh reads is hidden under this step's matmuls
                        s = (j // 8) % 4
                        seed_mms(1 - q, x_ring[:, s * 512:(s + 1) * 512])
                    if j % 8 == 4:
                        # prefetch x for the seed group after the next one
                        s_next = (j // 8 + 1) % 4
                        gg = g * ((U // 8) * 256) + (j // 8 + 2) * 256
                        fill_seed_rhs(s_next * 512, x_d[:, bass.ds(gg, 256)])

                    # h_preT[o] += sum_k Wt(k,o).T @ hT_k
                    # Chunk pairs (2a, 2a+1) share a bank. Pair emission is
                    # rotated by step and each pair consumes hT chunks in the
                    # previous step's production order, so every cross-step
                    # ACT->matmul edge has >= ~16 matmul slots of slack and
                    # PE never stalls on tanh completion.
                    k_order = [
                        2 * ((j - 1 + i2) % 4) + h2
                        for i2 in range(4) for h2 in range(2)
                    ]
                    for i in range(4):
                        a = (j + i) % 4
                        bank = ps[q * 4 + a]
                        for k in k_order:
                            for half in range(2):
                                o = 2 * a + half
                                nc.tensor.matmul(
                                    bank[:, half * 256 + r * 32:half * 256 + r * 32 + 32],
                                    w_sb[:, (k * NCH + o) * P:(k * NCH + o + 1) * P],
                                    h_in[:, k * 32:(k + 1) * 32],
                                    start=False,
                                    stop=(k == k_order[-1]),
                                    skip_group_check=True,
                                )
                        # tanh the whole pair straight out of psum
                        nc.scalar.activation(
                            h_out[:].rearrange("p (o b) -> p o b", b=B)[
                                :, 2 * a:2 * a + 2, :],
                            bank[:].rearrange("p (h rb) -> p h rb", h=2)[
                                :, :, r * 32:r * 32 + 32],
                            mybir.ActivationFunctionType.Tanh,
                        )

                    # z[p, b] = sum_o wout[p, o*32+b] * h_out[p, o*32+b]
                    # (mult fully contiguous; only the reduce is strided)
                    nc.vector.tensor_mul(prod[:], wout_sb[:], h_out[:])
                    nc.vector.tensor_reduce(
                        z_ring[:, (j % 64) * 32:(j % 64) * 32 + 32],
                        prod[:].rearrange("p (o b) -> p b o", o=NCH),
                        axis=mybir.AxisListType.X,
                        op=mybir.AluOpType.add,
                    )

                    if j % 16 == 15:
                        zslot = ((j // 16) % 4) * 512
                        nc.sync.dma_start(
                            out=zbuf_d[:, bass.ds(g * (U * B) + (j // 16) * 512, 512)],
                            in_=z_ring[:, zslot:zslot + 512],
                        )

            # ---- epilogue: y[t*32+b] = sum_p zbuf[p, t*32+b] ----
            nchunks = t_total * B // 512
            for c in range(nchunks):
                slot = c % 4
                nc.sync.dma_start(
                    out=z_ring[:, slot * 512:(slot + 1) * 512],
                    in_=zbuf_d[:, c * 512:(c + 1) * 512],
                )
                nc.tensor.matmul(
                    ps[c % 8][0:1, 0:512],
                    ones_sb[:, 0:1],
                    z_ring[:, slot * 512:(slot + 1) * 512],
                    start=True,
                    stop=True,
                )
                yslot = c % 4
                if c % 2 == 0:
                    nc.vector.tensor_copy(
                        y_sb[0:1, yslot * 512:(yslot + 1) * 512],
                        ps[c % 8][0:1, 0:512],
                    )
                else:
                    nc.scalar.copy(
                        y_sb[0:1, yslot * 512:(yslot + 1) * 512],
                        ps[c % 8][0:1, 0:512],
                    )
                nc.sync.dma_start(
                    out=y_d[0:1, c * 512:(c + 1) * 512],
                    in_=y_sb[0:1, yslot * 512:(yslot + 1) * 512],
                )

    nc.compile()
    _CACHE[key] = nc
    return nc


def _prep_inputs(initial_input, hidden, targets, W_ih, W_hh, b_ih, b_hh,
                 W_out, t_total):
    """Host-side prep: returns the 8 per-core input maps."""
    f32 = np.float32
    # x sequence: teacher-forced input at step t is targets[t-1], x_0 = initial
    x_seq = np.concatenate(
        [np.asarray(initial_input, f32).reshape(1, -1),
         np.asarray(targets, f32)[: t_total - 1, :, 0]],
        axis=0,
    )  # [T, 256]
    c_bias = (np.asarray(b_ih, f32) + np.asarray(b_hh, f32))
    # w_sb[p, (k*8+o)*128+m] = W_hh.T[k*128+p, o*128+m]
    wt = (
        np.asarray(W_hh, f32).T.reshape(NCH, P, NCH, P)
        .transpose(1, 0, 2, 3)
        .reshape(P, NCH * NCH * P)
        .astype(NP_MAIN)
    )
    # seedw[opar*2+t, a*128+m] = [W_ih; c][t] at H-index (2a+opar)*128+m
    sw = np.stack(
        [np.asarray(W_ih, f32)[:, 0].reshape(NCH, P), c_bias.reshape(NCH, P)],
        axis=1,
    )  # [o, t, m]
    seedw = (
        sw.reshape(4, 2, 2, P).transpose(1, 2, 0, 3).reshape(4, 4 * P)
        .astype(NP_MAIN)
    )
    wout8 = np.asarray(W_out, f32)[0].reshape(NCH, P).T                  # [128, 8]
    wout = np.ascontiguousarray(
        np.broadcast_to(wout8[:, :, None], (P, NCH, B)).reshape(P, NCH * B)
    )

    in_maps = []
    for ci in range(N_CORES):
        sl = slice(ci * B, (ci + 1) * B)
        xpad = np.zeros((2, t_total * B + 512), NP_MAIN)
        xpad[0, : t_total * B] = x_seq[:, sl].reshape(-1).astype(NP_MAIN)
        xpad[1, :] = 1.0
        h0 = np.ascontiguousarray(np.asarray(hidden, f32)[sl].T).astype(NP_MAIN)
        in_maps.append({
            "w": wt, "seedw": seedw, "x": xpad, "h0": h0, "wout": wout,
        })
    return in_maps


def kernel(initial_input, hidden, targets, W_ih, W_hh, b_ih, b_hh, W_out,
           b_out, teacher_force_probability=None, _trace=False):
    t_total = int(np.asarray(targets).shape[0])
    nc = _build(t_total, debug=False)
    in_maps = _prep_inputs(initial_input, hidden, targets, W_ih, W_hh, b_ih,
                           b_hh, W_out, t_total)
    res = run_bass_kernel_spmd(nc, in_maps, core_ids=list(range(N_CORES)),
                               trace=_trace)
    # y_core[0, t*32+b]; global batch index = core*32 + b
    y = np.concatenate(
        [r["y"].reshape(t_total, B) for r in res.results], axis=1
    ).astype(np.float32)
    y = y + np.float32(np.asarray(b_out).reshape(-1)[0])
    out = y[:, :, None]
    if _trace:
        return out, res
    return out

